# revision 3
# baseline (speedup 1.0000x reference)
"""Multi-head causal attention (B=8, S=1024, E=512, H=8, Dk=Dv=64) on 8 NeuronCores.

Sharding: data-parallel over batch. Core b computes the full transformer
attention block for X[b]; no collectives. Host pre-transposes X[b] -> [E, S]
and pre-arranges weights so the device kernel is pure matmul + softmax.

Device-side dataflow (per core, all matmuls float32r at full PE rate):
  XT [E,S] resident in SBUF (4 tiles of [128, 1024])
  V  = (X @ Wv + bv)            -> 8 tiles [128 s, 512 hd]
  QT = (X @ Wq + bq)^T          -> per head-pair tiles [128 dd, 512 q] (x2 q-halves)
  KT likewise
  per head h, q-chunk qc (512 cols):
    scores^T blocks [128 k, 512 q] = KT_h-block^T-matmul, causal mask added from
    a precomputed staircase slab, exp on ScalarE (scale=1/8 folded in),
    O^T accum = V-slice^T @ exp-blocks,  denom = ones^T @ exp-blocks,
    O^T *= 1/denom
  Y[s-chunk] = sum_h O_h^T-block^T @ Wo_h + bo
"""

import numpy as np

import concourse.bass as bass
import concourse.tile as tile
import concourse.mybir as mybir
from concourse import bacc
from concourse import bass_utils

B, S, E = 8, 1024, 512
H, DK, DV = 8, 64, 64
HD = H * DK  # 512
P = 128
EC = E // P  # 4 contraction chunks over E
NPAIR = H // 2  # head pairs packed on partitions
QCN = S // 512  # 2 q-chunks of 512
NCORES = 8
F32 = mybir.dt.float32
F32R = mybir.dt.float32r
NEG = -1.0e9

_COMPILED = None


def _r(ap):
    return ap.bitcast(F32R)


def _body(nc, tc, const, work, ps, pb, d):
    # ---- load constants / inputs into SBUF ----
    xt = []
    for c in range(EC):
        t = const.tile([P, S], F32R, tag=f"xt{c}", name=f"xt{c}")
        nc.sync.dma_start(t[:], d["xt"][c * P:(c + 1) * P, :])
        xt.append(t)
    w_sb = {}
    for wname in ("wq", "wk", "wv"):
        tiles = []
        for c in range(EC):
            t = const.tile([P, HD], F32R, tag=f"{wname}{c}", name=f"{wname}{c}")
            nc.sync.dma_start(t[:], d[wname][c * P:(c + 1) * P, :])
            tiles.append(t)
        w_sb[wname] = tiles
    woh = []
    for h in range(H):
        t = const.tile([DV, E], F32R, tag=f"woh{h}", name=f"woh{h}")
        nc.sync.dma_start(t[:], d["woh"][h])
        woh.append(t)
    mask_t = const.tile([P, 512], F32, tag="mask", name="mask_t")
    nc.sync.dma_start(mask_t[:], d["mask"][:])
    ones_t = const.tile([P, P], F32R, tag="ones", name="ones_t")
    nc.sync.dma_start(ones_t[:], d["ones"][:])
    bq_t = const.tile([P, NPAIR], F32, tag="bq", name="bq_t")
    nc.sync.dma_start(bq_t[:], d["bq"][:])
    bk_t = const.tile([P, NPAIR], F32, tag="bk", name="bk_t")
    nc.sync.dma_start(bk_t[:], d["bk"][:])
    bvb_t = const.tile([P, HD], F32, tag="bvb", name="bvb_t")
    nc.sync.dma_start(bvb_t[:], d["bvb"][:])
    bob_t = const.tile([P, E], F32, tag="bob", name="bob_t")
    nc.sync.dma_start(bob_t[:], d["bob"][:])

    # ---- V = X @ Wv + bv : per s-chunk [128 s, 512 hd] ----
    v_sb = []
    for si in range(S // P):
        vp = ps.tile([P, HD], F32, tag="ps512", name=f"vp{si}")
        for c in range(EC):
            nc.tensor.matmul(
                vp[:], _r(xt[c][:, si * P:(si + 1) * P]), _r(w_sb["wv"][c][:]),
                start=(c == 0), stop=(c == EC - 1))
        t = const.tile([P, HD], F32R, tag=f"v{si}", name=f"v{si}")
        nc.vector.tensor_add(t[:], vp[:], bvb_t[:])
        v_sb.append(t)

    # ---- QT / KT per head-pair, per q-half: [128 dd, 512 s] ----
    qt = {}
    kt = {}
    for p in range(NPAIR):
        for qc in range(QCN):
            qp = ps.tile([P, 512], F32, tag="ps512", name=f"qtp{p}_{qc}")
            for c in range(EC):
                nc.tensor.matmul(
                    qp[:], _r(w_sb["wq"][c][:, p * P:(p + 1) * P]),
                    _r(xt[c][:, qc * 512:(qc + 1) * 512]),
                    start=(c == 0), stop=(c == EC - 1))
            t = const.tile([P, 512], F32R, tag=f"qt{p}_{qc}", name=f"qt{p}_{qc}")
            nc.vector.tensor_scalar_add(t[:], qp[:], bq_t[:, p:p + 1])
            qt[p, qc] = t

            kp = ps.tile([P, 512], F32, tag="ps512", name=f"ktp{p}_{qc}")
            for c in range(EC):
                nc.tensor.matmul(
                    kp[:], _r(w_sb["wk"][c][:, p * P:(p + 1) * P]),
                    _r(xt[c][:, qc * 512:(qc + 1) * 512]),
                    start=(c == 0), stop=(c == EC - 1))
            t = const.tile([P, 512], F32R, tag=f"kt{p}_{qc}", name=f"kt{p}_{qc}")
            nc.vector.tensor_scalar_add(t[:], kp[:], bk_t[:, p:p + 1])
            kt[p, qc] = t

    # ---- attention per head, per q-chunk ----
    ot_sb = {}
    for h in range(H):
        p, hb = h // 2, h % 2
        hp = slice(hb * DK, (hb + 1) * DK)  # head's rows within pair tiles
        for qc in range(QCN):
            n_ki = 4 * (qc + 1)  # causal: only k-blocks with ki*128 <= qc*512+511
            otp = ps.tile([DV, 512], F32, tag="ps512", name=f"otp{h}_{qc}")
            smp = ps.tile([P, 512], F32, tag="ps512", name=f"smp{h}_{qc}")
            for g in range(n_ki // 2):
                stp = pb.tile([P, 1024], F32, tag="st", name=f"st{h}_{qc}_{g}")
                for j in range(2):
                    ki = 2 * g + j
                    kc, kl = ki // 4, ki % 4
                    nc.tensor.matmul(
                        stp[:, j * 512:(j + 1) * 512],
                        _r(kt[p, kc][hp, kl * P:(kl + 1) * P]),
                        _r(qt[p, qc][hp, :]),
                        start=True, stop=True)
                    off = ki * P - qc * 512
                    if off >= 0:
                        w = off + P  # mask region always ends at column 512
                        nc.vector.tensor_add(
                            stp[:, j * 512:j * 512 + w],
                            stp[:, j * 512:j * 512 + w],
                            mask_t[:, 512 - w:512])
                ste = work.tile([P, 1024], F32R, tag="ste", name=f"ste{h}_{qc}_{g}")
                nc.scalar.activation(
                    ste[:], stp[:], mybir.ActivationFunctionType.Exp, scale=0.125)
                for j in range(2):
                    ki = 2 * g + j
                    nc.tensor.matmul(
                        otp[:], _r(v_sb[ki][:, h * DV:(h + 1) * DV]),
                        _r(ste[:, j * 512:(j + 1) * 512]),
                        start=(ki == 0), stop=(ki == n_ki - 1))
                    nc.tensor.matmul(
                        smp[:], _r(ones_t[:]),
                        _r(ste[:, j * 512:(j + 1) * 512]),
                        start=(ki == 0), stop=(ki == n_ki - 1))
            rec = work.tile([DV, 512], F32, tag="rec", name=f"rec{h}_{qc}", bufs=2)
            nc.vector.reciprocal(rec[:], smp[0:DV, :])
            ot = const.tile([DV, 512], F32R, tag=f"ot{h}_{qc}", name=f"ot{h}_{qc}")
            nc.vector.tensor_mul(ot[:], otp[:], rec[:])
            ot_sb[h, qc] = ot

    # ---- output projection Y[s-chunk] = sum_h OT_h-block^T @ Wo_h + bo ----
    for si in range(S // P):
        qc, sl = si // 4, si % 4
        yp = ps.tile([P, E], F32, tag="ps512", name=f"yp{si}")
        for h in range(H):
            nc.tensor.matmul(
                yp[:], _r(ot_sb[h, qc][:, sl * P:(sl + 1) * P]), _r(woh[h][:]),
                start=(h == 0), stop=(h == H - 1))
        yo = work.tile([P, E], F32, tag="yo", name=f"yo{si}", bufs=2)
        nc.vector.tensor_add(yo[:], yp[:], bob_t[:])
        nc.sync.dma_start(d["y"][si * P:(si + 1) * P, :], yo[:])


def _build():
    nc = bacc.Bacc("TRN2", target_bir_lowering=False, debug=False)
    d = {
        "xt": nc.dram_tensor("xt", [E, S], F32R, kind="ExternalInput").ap(),
        "wq": nc.dram_tensor("wq", [E, HD], F32R, kind="ExternalInput").ap(),
        "wk": nc.dram_tensor("wk", [E, HD], F32R, kind="ExternalInput").ap(),
        "wv": nc.dram_tensor("wv", [E, HD], F32R, kind="ExternalInput").ap(),
        "woh": nc.dram_tensor("woh", [H, DV, E], F32R, kind="ExternalInput").ap(),
        "mask": nc.dram_tensor("mask", [P, 512], F32, kind="ExternalInput").ap(),
        "ones": nc.dram_tensor("ones", [P, P], F32R, kind="ExternalInput").ap(),
        "bq": nc.dram_tensor("bq", [P, NPAIR], F32, kind="ExternalInput").ap(),
        "bk": nc.dram_tensor("bk", [P, NPAIR], F32, kind="ExternalInput").ap(),
        "bvb": nc.dram_tensor("bvb", [P, HD], F32, kind="ExternalInput").ap(),
        "bob": nc.dram_tensor("bob", [P, E], F32, kind="ExternalInput").ap(),
        "y": nc.dram_tensor("y", [S, E], F32, kind="ExternalOutput").ap(),
    }
    with tile.TileContext(nc) as tc:
        with tc.tile_pool(name="const", bufs=1) as const, \
             tc.tile_pool(name="work", bufs=3) as work, \
             tc.tile_pool(name="ps", bufs=4, space="PSUM") as ps, \
             tc.tile_pool(name="pb", bufs=2, space="PSUM") as pb:
            _body(nc, tc, const, work, ps, pb, d)
    nc.compile()
    return nc


def get_nc():
    global _COMPILED
    if _COMPILED is None:
        _COMPILED = _build()
    return _COMPILED


def _prep_in_maps(X, Wq, bq, Wk, bk, Wv, bv, Wo, bo):
    f = np.float32
    shared = {
        "wq": np.ascontiguousarray(np.transpose(np.asarray(Wq, f), (1, 0, 2)).reshape(E, HD)),
        "wk": np.ascontiguousarray(np.transpose(np.asarray(Wk, f), (1, 0, 2)).reshape(E, HD)),
        "wv": np.ascontiguousarray(np.transpose(np.asarray(Wv, f), (1, 0, 2)).reshape(E, HD)),
        "woh": np.ascontiguousarray(np.asarray(Wo, f).reshape(H, DV, E)),
        "bq": np.ascontiguousarray(np.asarray(bq, f).reshape(HD).reshape(NPAIR, P).T),
        "bk": np.ascontiguousarray(np.asarray(bk, f).reshape(HD).reshape(NPAIR, P).T),
        "bvb": np.ascontiguousarray(np.broadcast_to(np.asarray(bv, f).reshape(1, HD), (P, HD))),
        "bob": np.ascontiguousarray(np.broadcast_to(np.asarray(bo, f).reshape(1, E), (P, E))),
        "ones": np.ones((P, P), f),
    }
    # staircase causal mask slab: M[k, j] = NEG where k > j - 384 (j in [0,512)).
    # block (ki, qc) with off = ki*128 - qc*512 >= 0 uses columns [384-off, 512).
    kk = np.arange(P)[:, None]
    jj = np.arange(512)[None, :]
    shared["mask"] = np.where(kk > jj - 384, f(NEG), f(0.0)).astype(f)
    Xf = np.asarray(X, f)
    in_maps = []
    for b in range(B):
        m = dict(shared)
        m["xt"] = np.ascontiguousarray(Xf[b].T)
        in_maps.append(m)
    return in_maps


def kernel(X, Wq, bq, Wk, bk, Wv, bv, Wo, bo):
    nc = get_nc()
    in_maps = _prep_in_maps(X, Wq, bq, Wk, bk, Wv, bv, Wo, bo)
    res = bass_utils.run_bass_kernel_spmd(nc, in_maps, core_ids=list(range(NCORES)))
    return np.stack([res.results[b]["y"] for b in range(B)], axis=0).astype(np.float32)


def run_traced(X, Wq, bq, Wk, bk, Wv, bv, Wo, bo):
    """Like kernel() but with NTFF profiling; returns (out, exec_time_ns)."""
    nc = get_nc()
    in_maps = _prep_in_maps(X, Wq, bq, Wk, bk, Wv, bv, Wo, bo)
    res = bass_utils.run_bass_kernel_spmd(
        nc, in_maps, core_ids=list(range(NCORES)), trace=True)
    out = np.stack([res.results[b]["y"] for b in range(B)], axis=0).astype(np.float32)
    return out, res.exec_time_ns
